# revision 27
# baseline (speedup 1.0000x reference)
"""Trainium2 Bass kernel for DiceLoss (hard-argmax dice, ignore background, mean).

Problem (hardcoded shapes):
  y_true: [16, 512, 512] int32 in [0, 8)
  y_pred: [16, 8, 512, 512] float32
  out   : scalar float32 = mean over classes 1..7 of
          (2*tp + eps) / (2*tp + fp + fn + eps)

Strategy v4 (8 NeuronCores, 2 images per core):
  ALL loads ride HWDGE. Earlier versions cast f32->bf16 in SWDGE
  DMAs, but SWDGE descriptor writing runs on the GpSimd Q7 cores and
  is ring-paced across the whole stream -- and DVE tensor_tensor ops
  (2x mode) use the shared SBUF port for src1, so Q7's descriptor
  writes and DVE's TT ops throttled EACH OTHER (~+300 cycles/op,
  stream stalls). HWDGE descriptor generation is RTL: zero Q7
  activity, zero port coupling. ScalarE (otherwise idle) does all
  f32/int32 -> bf16 conversion, trailing each chunk's arrival by ~2us.

  Work is split into 5 column sections -- img0 (0:1024), (1024:2048),
  img1 (0:1024), (1024:1536), (1536:2048) -- each loaded as 4
  channel-pair chunks. Per section: 4 pmax + 3 chain ops give the max,
  then pred masks. This spreads pred work across the stream (v2/v3's
  half-plane split piled 17us of preds after stream end). The LAST
  section's pmax ops read the f32 staging directly (TT 1x, bf16 out)
  so the tail max never waits on a convert; its preds use ScalarE
  conversions that complete in parallel.

  DVE op shapes: gt masks as 7 two-image ops (TS is_equal 4x) into ONE
  gt tile [P, 2, 7, NSUB, 129] (col 128 = ones); preds as
  channel-pair-merged TT is_equal against a stride-0-broadcast m.

  TensorE per class: psum bank [P, 387] = [G 0:258 | tp 258:387].
  G: lhsT=ones128, rhs=gt[:, both-images, c, s, :], 16 accums (rows
  identical = per-column gt sums), scheduled in PE's idle early window.
  tp/predcnt: lhsT=pred subtile, rhs=gt[n, c, s, 0:129], 32 accums
  (col 128 = pred counts).

  ScalarE: converts + G row-0 evacs (mid-stream) + tp evacs (tail).
  Host: tp = trace, pred_cnt = col-128 sums, gt_cnt = G row-0 sums.
"""

import numpy as np

EPS = 1e-05

N_CORES = 8
NB = 2
C = 8
P = 128
FD = 2048
NSUB = FD // 128

# (image, col offset, col length)
SECS = [(0, 0, 1024), (0, 1024, 1024), (1, 0, 1024), (1, 1024, 512), (1, 1536, 512)]

_CACHED_NC = None


def build_bass():
    from contextlib import ExitStack

    import concourse.bacc as bacc
    import concourse.tile as tile
    from concourse import mybir

    nc = bacc.Bacc(None, target_bir_lowering=False)

    yp = nc.dram_tensor("yp", [NB, C, P, FD], mybir.dt.float32, kind="ExternalInput")
    yt = nc.dram_tensor("yt", [NB, P, FD], mybir.dt.int32, kind="ExternalInput")
    mm_out = nc.dram_tensor("mm_out", [7, P, 129], mybir.dt.float32, kind="ExternalOutput")
    g_out = nc.dram_tensor("g_out", [7, 258], mybir.dt.float32, kind="ExternalOutput")

    # arrival stamps (ms): HWDGE starts ~8.5us, ~2.6us/MiB read
    T_L = [0.0111, 0.0137]
    T_CHUNK = [
        [0.0163, 0.0189, 0.0215, 0.0241],   # img0 A
        [0.0267, 0.0293, 0.0319, 0.0345],   # img0 B
        [0.0371, 0.0397, 0.0423, 0.0449],   # img1 A
        [0.0462, 0.0475, 0.0488, 0.0501],   # img1 B1
        [0.0514, 0.0527, 0.0540, 0.0553],   # img1 B2
    ]
    CONV_LAG = 0.0021

    with tile.TileContext(nc) as tc, ExitStack() as ctx:
        chpool = ctx.enter_context(tc.tile_pool(name="ch", bufs=1))
        stgp = ctx.enter_context(tc.tile_pool(name="stg", bufs=4))
        mpool = ctx.enter_context(tc.tile_pool(name="mx", bufs=2))
        mtmp = ctx.enter_context(tc.tile_pool(name="mtmp", bufs=4))
        predp = ctx.enter_context(tc.tile_pool(name="pred", bufs=3))
        accp = ctx.enter_context(tc.tile_pool(name="acc", bufs=1))
        psump = ctx.enter_context(tc.tile_pool(name="psum", bufs=1, space="PSUM"))

        gtall = accp.tile([P, NB, 7, NSUB, 129], mybir.dt.bfloat16, name="gtall")
        ones128 = accp.tile([P, 128], mybir.dt.bfloat16, name="ones128")
        tfb = accp.tile([P, NB, FD], mybir.dt.bfloat16, name="tfb")
        psums = [
            psump.tile([P, 387], mybir.dt.float32, name=f"ps{c}", tag=f"ps{c}")
            for c in range(1, C)
        ]

        def g_ap(c):
            return psums[c - 1][:, 0:258]

        def tp_ap(c):
            return psums[c - 1][:, 258:387]

        nc.vector.memset(ones128, 1.0)
        nc.vector.memset(gtall[:, :, :, :, 128:129], 1.0)

        # ---- labels: HWDGE stage + ScalarE convert into tfb. (A SWDGE
        # cast into the tfb SLICE was 5x slower: the sliced dst explodes
        # into per-partition descriptors and Q7 ground for 14us.) ----
        for n in range(NB):
            st = stgp.tile([P, FD], mybir.dt.int32, name="stg", tag="stg")
            nc.sync.dma_start(out=st, in_=yt[n])
            with tc.tile_wait_until(T_L[n]):
                nc.scalar.copy(out=tfb[:, n, :], in_=st)

        # ---- yp chunks: HWDGE stage + ScalarE convert ----
        # chunks[sec][k] = bf16 tile [P, 2, ln] (channels 2k, 2k+1)
        chunks = []
        stgs = []
        for sec, (n, off, ln) in enumerate(SECS):
            row_c, row_s = [], []
            for k in range(4):
                st = stgp.tile([P, 2, ln], mybir.dt.float32, name="stg", tag="stg")
                nc.sync.dma_start(
                    out=st,
                    in_=yp[n, 2 * k : 2 * k + 2, :, off : off + ln].rearrange(
                        "c p x -> p c x"))
                bt = chpool.tile([P, 2, ln], mybir.dt.bfloat16,
                                 name=f"c{sec}_{k}", tag=f"c{sec}_{k}")
                with tc.tile_wait_until(T_CHUNK[sec][k] + CONV_LAG):
                    nc.scalar.copy(out=bt, in_=st)
                row_c.append(bt)
                row_s.append(st)
            chunks.append(row_c)
            stgs.append(row_s)

        def emit_gt(c, ts):
            tf4 = tfb[:].rearrange("p n (s f) -> p n s f", s=NSUB)
            with tc.tile_wait_until(ts):
                nc.vector.tensor_single_scalar(
                    out=gtall[:, :, c - 1, :, 0:128], in_=tf4,
                    scalar=float(c), op=mybir.AluOpType.is_equal,
                )

        def emit_gcnt(c, ts):
            for s in range(NSUB):
                with tc.tile_wait_until(ts):
                    nc.tensor.matmul(
                        psums[c - 1][0:1, 0:258], lhsT=ones128[:, 0:1],
                        rhs=gtall[:, :, c - 1, s, :],
                        start=(s == 0), stop=(s == NSUB - 1),
                    )

        def emit_tree(sec, ln, from_f32):
            """Non-tail sections: 4-op merged tree -- level1 maxes whole
            chunk tiles elementwise (pairing channels (0,2),(1,3) etc.,
            which a max doesn't care about), halving DVE op count.
            Tail section: per-chunk pmax + serial chain reading the f32
            staging directly, so only 2 small ops trail the last byte."""
            gates = T_CHUNK[sec] if from_f32 else [t + CONV_LAG + 0.001
                                                   for t in T_CHUNK[sec]]
            m = mpool.tile([P, 1024], mybir.dt.bfloat16, name="m", tag="m")
            if not from_f32:
                q01 = mtmp.tile([P, 2, 1024], mybir.dt.bfloat16, name="q01", tag="mt")
                with tc.tile_wait_until(gates[1]):
                    nc.vector.tensor_max(
                        q01[:, :, 0:ln], chunks[sec][0][:], chunks[sec][1][:])
                q23 = mtmp.tile([P, 2, 1024], mybir.dt.bfloat16, name="q23", tag="mt")
                with tc.tile_wait_until(gates[3]):
                    nc.vector.tensor_max(
                        q23[:, :, 0:ln], chunks[sec][2][:], chunks[sec][3][:])
                e = mtmp.tile([P, 2, 1024], mybir.dt.bfloat16, name="e", tag="mt")
                with tc.tile_wait_until(gates[3]):
                    nc.vector.tensor_max(e[:, :, 0:ln], q01[:, :, 0:ln], q23[:, :, 0:ln])
                    nc.vector.tensor_max(m[:, 0:ln], e[:, 0, 0:ln], e[:, 1, 0:ln])
                return m
            # serial f32 chain over staging chunk tiles: only two ops
            # (the last accumulate + the 2-channel fold) trail the final
            # byte, and nothing waits on a convert.
            acc = None
            for k in range(1, 4):
                t = mtmp.tile([P, 2, 512], mybir.dt.float32, name=f"a{k}", tag="mt")
                with tc.tile_wait_until(gates[k]):
                    nc.vector.tensor_max(
                        t[:], stgs[sec][k][:], acc if acc is not None
                        else stgs[sec][0][:])
                acc = t[:]
            with tc.tile_wait_until(gates[3]):
                nc.vector.tensor_max(m[:, 0:ln], acc[:, 0, :], acc[:, 1, :])
            return m

        def emit_preds_mm(sec, n, off, ln, m):
            """pred masks: c1 single, then channel-pair-merged ops with
            broadcast m; tp matmuls per class."""
            ns = ln // 128
            s0 = off // 128
            first, last = (sec == 0), (sec == len(SECS) - 1)
            ts = (T_CHUNK[sec][3] if sec == len(SECS) - 1
                  else T_CHUNK[sec][3] + CONV_LAG) + 0.0012
            mb = m[:, 0:ln].rearrange("p (o x) -> p o x", o=1).broadcast_to(
                [P, 2, ln])

            def mm(c, predv):
                for s in range(ns):
                    nc.tensor.matmul(
                        tp_ap(c),
                        lhsT=predv[:, s * 128 : (s + 1) * 128],
                        rhs=gtall[:, n, c - 1, s0 + s, :],
                        start=(first and s == 0),
                        stop=(last and s == ns - 1),
                    )

            p1 = predp.tile([P, 2, 1024], mybir.dt.bfloat16, name="p1", tag="pred")
            with tc.tile_wait_until(ts):
                nc.vector.tensor_tensor(
                    out=p1[:, 0, 0:ln], in0=chunks[sec][0][:, 1, :],
                    in1=m[:, 0:ln], op=mybir.AluOpType.is_equal)
            mm(1, p1[:, 0, 0:ln])
            for k in (1, 2, 3):
                pk = predp.tile([P, 2, 1024], mybir.dt.bfloat16, name=f"pk{k}", tag="pred")
                with tc.tile_wait_until(ts):
                    nc.vector.tensor_tensor(
                        out=pk[:, :, 0:ln], in0=chunks[sec][k],
                        in1=mb, op=mybir.AluOpType.is_equal)
                mm(2 * k, pk[:, 0, 0:ln])
                mm(2 * k + 1, pk[:, 1, 0:ln])

        # DVE program
        for c in range(1, C):
            emit_gt(c, T_L[1] + CONV_LAG)
        for c in range(1, C):
            emit_gcnt(c, T_L[1] + CONV_LAG + 0.008)

        # G evacs: parked at the END of ScalarE's in-order queue (after
        # all converts) -- anywhere earlier they head-of-line block the
        # converts that pace the staging WAR chain
        evg = accp.tile([1, 7, 258], mybir.dt.float32, name="evg")
        for c in range(1, C):
            with tc.tile_wait_until(0.0575 + 0.0002 * (c - 1)):
                nc.scalar.copy(out=evg[:, c - 1, :], in_=psums[c - 1][0:1, 0:258])
        nc.sync.dma_start(out=g_out[:], in_=evg)

        for sec, (n, off, ln) in enumerate(SECS):
            m = emit_tree(sec, ln, from_f32=(sec == len(SECS) - 1))
            emit_preds_mm(sec, n, off, ln, m)

        pt = accp.tile([P, 7, 129], mybir.dt.float32, name="pt")
        for c in range(1, C):
            nc.scalar.copy(out=pt[:, c - 1, :], in_=psums[c - 1][:, 258:387])
            nc.sync.dma_start(out=mm_out[c - 1], in_=pt[:, c - 1, :])

    nc.finalize()
    return nc


def _get_bass():
    global _CACHED_NC
    if _CACHED_NC is None:
        _CACHED_NC = build_bass()
    return _CACHED_NC


def make_in_maps(y_true, y_pred):
    yp = np.ascontiguousarray(np.asarray(y_pred, dtype=np.float32))
    yt = np.ascontiguousarray(np.asarray(y_true, dtype=np.int32))
    in_maps = []
    for i in range(N_CORES):
        yps = np.ascontiguousarray(yp[NB * i : NB * (i + 1)]).reshape(NB, C, P, FD)
        yts = np.ascontiguousarray(yt[NB * i : NB * (i + 1)]).reshape(NB, P, FD)
        in_maps.append({"yp": yps, "yt": yts})
    return in_maps


def epilogue(results):
    tp = np.zeros(7, dtype=np.float64)
    pred_cnt = np.zeros(7, dtype=np.float64)
    gt_cnt = np.zeros(7, dtype=np.float64)
    for r in results:
        mm = np.asarray(r["mm_out"], dtype=np.float64)  # [7, P, 129]
        tp += np.trace(mm[:, :, 0:128], axis1=1, axis2=2)
        pred_cnt += mm[:, :, 128].sum(axis=1)
        g = np.asarray(r["g_out"], dtype=np.float64)    # [7, 258]
        gt_cnt += g[:, 0:128].sum(axis=1) + g[:, 129:257].sum(axis=1)

    tp32 = tp.astype(np.float32)
    fp32_ = (pred_cnt - tp).astype(np.float32)
    fn32 = (gt_cnt - tp).astype(np.float32)
    eps = np.float32(EPS)
    two = np.float32(2.0)
    dice = (two * tp32 + eps) / (two * tp32 + fp32_ + fn32 + eps)
    return np.asarray(np.mean(dice, dtype=np.float32), dtype=np.float32)


def kernel(**inputs):
    from concourse.bass_utils import run_bass_kernel_spmd

    nc = _get_bass()
    in_maps = make_in_maps(inputs["y_true"], inputs["y_pred"])
    res = run_bass_kernel_spmd(nc, in_maps, core_ids=list(range(N_CORES)))
    return epilogue(res.results)


if __name__ == "__main__":
    rng = np.random.default_rng(0)
    y_true = rng.integers(0, C, size=(16, 512, 512)).astype(np.int32)
    y_pred = rng.standard_normal((16, C, 512, 512)).astype(np.float32)
    out = kernel(y_true=y_true, y_pred=y_pred)
    print("kernel output:", out)


# revision 31
# speedup vs baseline: 1.1095x; 1.1095x over previous
"""Trainium2 Bass kernel for DiceLoss (hard-argmax dice, ignore background, mean).

Problem (hardcoded shapes):
  y_true: [16, 512, 512] int32 in [0, 8)
  y_pred: [16, 8, 512, 512] float32
  out   : scalar float32 = mean over classes 1..7 of
          (2*tp + eps) / (2*tp + fp + fn + eps)
  with pred_cls = argmax_c y_pred, one-hot tp/fp/fn sums over all pixels.

Strategy (8 NeuronCores, data-parallel over batch; 2 images per core):
  - Each channel plane is one [128, 2048] tile. y_pred is loaded via SWDGE
    cast-DMA (f32 in HBM -> bf16 in SBUF): HBM read traffic is unchanged but
    every on-chip elementwise op runs in DVE 16-bit perf modes and no
    convert instructions are needed. The per-core stream is a single SWDGE
    FIFO at the HBM bandwidth limit, so everything else is ordered around
    its arrival times: image 0 loads whole planes; image 1 loads two half
    planes so only half a plane of compute trails the final DMA completion.
  - DVE (all bf16, no accum_out so the 2x/4x perf-mode uops stay eligible):
      * 7-op pairwise max tree -> m = max over channels      (2x_1P)
      * pred_c = (ch_c == m) via tensor_tensor is_equal      (2x_1P)
      * gt_c   = (tf == c) via tensor_single_scalar is_equal (4x_2P),
        written strided into a [128, 16, 130] block layout whose col 128
        holds a persistent ones column (memset once). Separate gt tile
        sets per image so image 1's writes never wait on image 0's matmul
        readers (WAR convoy).
  - ScalarE: int32->bf16 label convert; per section a flat copy-with-
    accum_out over the gt block layout that yields the per-partition gt
    counts (host subtracts the constant ones contribution); PSUM evac.
  - TensorE: per class-subtile one matmul with lhsT = pred subtile and
    rhs = [gt subtile | ones] (129 cols) accumulated over subtiles+images:
    diag gives tp, column 128 gives pred counts. Host reads trace + sums.
  - Host: combines the 8 cores' exact-integer f32 partials; dice needs only
    tp and pred_cnt+gt_cnt (denominator = 2tp+fp+fn = pred+gt), formed in
    float32 to match the reference arithmetic.
"""

import numpy as np

EPS = 1e-05

# Problem geometry (hardcoded per the harness contract).
N_CORES = 8
NB = 2            # batch images per core
C = 8             # classes
P = 128           # SBUF partitions
FD = 2048         # free-dim elements per channel plane (512*512 = 128*2048)
NSUB = FD // 128  # 128-wide subtiles per plane for the PE matmuls
BLK = 130         # gt block stride: 128 gt cols + ones col + 1 pad (4B align)

_CACHED_NC = None


def build_bass():
    """Build the Bass kernel (same NEFF for all 8 cores)."""
    from contextlib import ExitStack

    import concourse.bacc as bacc
    import concourse.tile as tile
    from concourse import mybir

    nc = bacc.Bacc(None, target_bir_lowering=False)

    yp = nc.dram_tensor("yp", [NB, C, P, FD], mybir.dt.float32, kind="ExternalInput")
    yt = nc.dram_tensor("yt", [NB, P, FD], mybir.dt.int32, kind="ExternalInput")
    # per class: [128, 129] PSUM accumulator (cross-products + pred colsum).
    mm_out = nc.dram_tensor("mm_out", [7, P, 129], mybir.dt.float32, kind="ExternalOutput")
    # per-partition gt counts: slots 0..6 = (img0, class), 7..13 = (img1,
    # half 0, class), 14..20 = (img1, half 1, class)
    ga_out = nc.dram_tensor("ga_out", [P, 14], mybir.dt.float32, kind="ExternalOutput")

    with tile.TileContext(nc) as tc, ExitStack() as ctx:
        chpool = ctx.enter_context(tc.tile_pool(name="ch", bufs=1))
        tpool = ctx.enter_context(tc.tile_pool(name="tt", bufs=1))
        mpool = ctx.enter_context(tc.tile_pool(name="mx", bufs=2))
        mtmp = ctx.enter_context(tc.tile_pool(name="mtmp", bufs=6))
        predp = ctx.enter_context(tc.tile_pool(name="pred", bufs=5))
        scrp = ctx.enter_context(tc.tile_pool(name="scr", bufs=1))
        accp = ctx.enter_context(tc.tile_pool(name="acc", bufs=1))
        psump = ctx.enter_context(tc.tile_pool(name="psum", bufs=1, space="PSUM"))

        ga_acc = accp.tile([P, 14], mybir.dt.float32, name="ga_acc")
        # ONE gt tile per image in block layout [128, 7, 16, 130] (classes
        # outer, so each class's 16 subtile blocks stay contiguous for the
        # flat-copy count read): cols 0:128 = gt mask, col 128 = ones,
        # col 129 = zero pad. Separate tiles per image so image 1's gt
        # writes never wait on image 0's matmul readers. Merging kills 26
        # of the 28 head memsets, whose DVE activity overlapped Q7's first
        # SWDGE descriptor generation.
        gtsets = [
            accp.tile([P, 7, NSUB, BLK], mybir.dt.bfloat16, name=f"gt{n}")
            for n in range(NB)
        ]
        psums = [
            psump.tile([P, 129], mybir.dt.float32, name=f"ps{c}", tag=f"ps{c}")
            for c in range(1, C)
        ]

        for g in gtsets:
            nc.vector.memset(g[:, :, :, 128:129], 1.0)
            nc.vector.memset(g[:, :, :, 129:130], 0.0)

        HF = FD // 2   # half-plane free dim
        HS = NSUB // 2  # subtiles per half

        # ---- all loads up front: gpsimd queue delivers the casts FIFO in
        # exactly this order; labels ride the concurrent HWDGE queue.
        # Image 0 loads whole planes; image 1 loads half-planes (half-major)
        # so the tail after the last byte is only half a plane's compute. ----
        ch = {}   # (n, c) -> full-plane AP;  (1, c, h) -> half-plane AP
        tf = {}
        for n in range(NB):
            ti = tpool.tile([P, FD], mybir.dt.int32, name="ti", tag=f"ti{n}")
            nc.sync.dma_start(out=ti, in_=yt[n])
            # labels to bf16 (exact for 0..7) on ScalarE
            tfn = tpool.tile([P, FD], mybir.dt.bfloat16, name="tf", tag=f"tf{n}")
            nc.scalar.copy(out=tfn, in_=ti)
            tf[n] = tfn
        for c in range(C):
            tl = chpool.tile([P, FD], mybir.dt.bfloat16, name=f"ch{c}", tag=f"n0ch{c}")
            # SWDGE cast-DMA: f32 HBM -> bf16 SBUF
            nc.gpsimd.dma_start(out=tl, in_=yp[0, c])
            ch[0, c] = tl
        # image 1 sections: two half-planes (quarter-splitting the tail was
        # measured slower — the extra small DMAs lengthen the SWDGE FIFO
        # more than the shorter trailing compute chain saves).
        sections = [(0, HF), (HF, HF)]
        im1 = {}
        for c in range(C):
            im1[c] = chpool.tile([P, FD], mybir.dt.bfloat16, name=f"ch{c}", tag=f"n1ch{c}")
        for si, (off, ln) in enumerate(sections):
            for c in range(C):
                part = im1[c][:, off : off + ln]
                nc.gpsimd.dma_start(out=part, in_=yp[1, c][:, off : off + ln])
                ch[1, c, si] = part

        def emit_gt(g, slot, tfv, c):
            """gt mask (DVE 4x, full plane) + gt count (ScalarE flat copy
            w/ accum) for class c of one image's gt tile."""
            gv = g[:, c - 1, :, 0:128]
            nc.vector.tensor_single_scalar(
                out=gv, in_=tfv, scalar=float(c), op=mybir.AluOpType.is_equal
            )
            scr = scrp.tile([P, NSUB * BLK], mybir.dt.bfloat16, name="scr", tag="scr")
            # flat contiguous read (incl. ones + zero pad; host subtracts
            # the constant 16 per partition per block) keeps ScalarE fast
            nc.scalar.activation(
                out=scr[:, 0 : NSUB * BLK],
                in_=g[:, c - 1, :, :].rearrange("p s f -> p (s f)"),
                func=mybir.ActivationFunctionType.Copy,
                accum_out=ga_acc[:, slot : slot + 1],
            )

        def emit_tree(chs, fd, t0, dt):
            """Serial-chain max: the last-arriving channels join closest to
            the root so only 1-2 ops trail the final DMA. The tile_wait_until
            stamps tell the static scheduler the real DMA arrival times, so
            it slots the (early-ready) gt-mask ops into the wait windows
            instead of head-of-line blocking the DVE queue on the tree."""
            t1 = mtmp.tile([P, FD], mybir.dt.bfloat16, name="t1", tag="mt")
            with tc.tile_wait_until(t0 + 2 * dt):
                nc.vector.tensor_max(t1[:, 0:fd], chs[0], chs[1])
            t2 = mtmp.tile([P, FD], mybir.dt.bfloat16, name="t2", tag="mt")
            t12 = mtmp.tile([P, FD], mybir.dt.bfloat16, name="t12", tag="mt")
            with tc.tile_wait_until(t0 + 4 * dt):
                nc.vector.tensor_max(t2[:, 0:fd], chs[2], chs[3])
                nc.vector.tensor_max(t12[:, 0:fd], t1[:, 0:fd], t2[:, 0:fd])
            t3 = mtmp.tile([P, FD], mybir.dt.bfloat16, name="t3", tag="mt")
            t123 = mtmp.tile([P, FD], mybir.dt.bfloat16, name="t123", tag="mt")
            with tc.tile_wait_until(t0 + 6 * dt):
                nc.vector.tensor_max(t3[:, 0:fd], chs[4], chs[5])
                nc.vector.tensor_max(t123[:, 0:fd], t12[:, 0:fd], t3[:, 0:fd])
            t6 = mtmp.tile([P, FD], mybir.dt.bfloat16, name="t6", tag="mt")
            with tc.tile_wait_until(t0 + 7 * dt):
                nc.vector.tensor_max(t6[:, 0:fd], t123[:, 0:fd], chs[6])
            m = mpool.tile([P, FD], mybir.dt.bfloat16, name="m", tag="m")
            with tc.tile_wait_until(t0 + 8 * dt):
                nc.vector.tensor_max(m[:, 0:fd], t6[:, 0:fd], chs[7])
            return m

        def emit_pred_mm(gts, chv, m, c, s0, ns, start, stop):
            pred = predp.tile([P, FD], mybir.dt.bfloat16, name=f"pred{c}", tag="pred")
            predv = pred[:, 0 : ns * 128]
            nc.vector.tensor_tensor(
                out=predv, in0=chv, in1=m, op=mybir.AluOpType.is_equal
            )
            g = gts
            for s in range(ns):
                nc.tensor.matmul(
                    psums[c - 1][:, :],
                    lhsT=predv[:, s * 128 : (s + 1) * 128],
                    rhs=g[:, c - 1, s0 + s, 0:129],
                    start=(start and s == 0),
                    stop=(stop and s == ns - 1),
                )

        # ---- DVE program, ordered to match arrival times ----
        # All gt masks first: they depend only on the labels (arrive within
        # ~12us on the HWDGE queue) so DVE starts productive work ~6us
        # before the first y_pred channel lands.
        # image 0's gt masks run entirely on ScalarE: gt = relu(1 - |tf - c|)
        # with the gt count fused into the Relu's accum_out (which replaces
        # the separate count-copy, so ScalarE gains only one op per class
        # while DVE sheds all seven tensor_scalar ops).
        tf3_0 = tf[0].rearrange("p (s f) -> p s f", s=NSUB)
        for c in range(1, C):
            emit_gt(gtsets[0], c - 1, tf3_0, c)
        tf3_1 = tf[1].rearrange("p (s f) -> p s f", s=NSUB)
        for c in range(1, C):
            emit_gt(gtsets[1], 7 + (c - 1), tf3_1, c)

        # Measured SWDGE FIFO timing (ms): first bytes ~11.5us in, then one
        # 1 MiB full-plane cast every ~2.8us (half planes ~1.4us).
        T0 = 0.0115
        DT_FULL = 0.0028
        DT_HALF = 0.0014

        # image 0: tree, then pred+MM per class.
        m0 = emit_tree([ch[0, c] for c in range(C)], FD, T0 - DT_FULL, DT_FULL)
        for c in range(1, C):
            emit_pred_mm(gtsets[0], ch[0, c], m0, c, 0, NSUB, start=True, stop=False)

        # image 1, by section: tree + pred/MM.
        for si, (off, ln) in enumerate(sections):
            s0, ns = off // 128, ln // 128
            t0 = T0 + 8 * DT_FULL + si * 8 * DT_HALF - DT_HALF
            mh = emit_tree([ch[1, c, si] for c in range(C)], ln, t0, DT_HALF)
            for c in range(1, C):
                emit_pred_mm(
                    gtsets[1], ch[1, c, si], mh[:, 0:ln], c, s0, ns,
                    start=False, stop=(si == len(sections) - 1),
                )

        nc.sync.dma_start(out=ga_out[:], in_=ga_acc)
        for c in range(7):
            pt = accp.tile([P, 129], mybir.dt.float32, name=f"pt{c}", tag=f"pt{c}")
            nc.scalar.copy(out=pt, in_=psums[c])
            nc.sync.dma_start(out=mm_out[c], in_=pt)

    nc.finalize()
    return nc


def _get_bass():
    global _CACHED_NC
    if _CACHED_NC is None:
        _CACHED_NC = build_bass()
    return _CACHED_NC


def make_in_maps(y_true, y_pred):
    yp = np.ascontiguousarray(np.asarray(y_pred, dtype=np.float32))
    yt = np.ascontiguousarray(np.asarray(y_true, dtype=np.int32))
    in_maps = []
    for i in range(N_CORES):
        yps = np.ascontiguousarray(yp[NB * i : NB * (i + 1)]).reshape(NB, C, P, FD)
        yts = np.ascontiguousarray(yt[NB * i : NB * (i + 1)]).reshape(NB, P, FD)
        in_maps.append({"yp": yps, "yt": yts})
    return in_maps


def epilogue(results):
    """Combine the 8 cores' partial sums into the final dice mean (float32,
    mirroring the reference arithmetic)."""
    tp = np.zeros(7, dtype=np.float64)
    pred_cnt = np.zeros(7, dtype=np.float64)
    gt_cnt = np.zeros(7, dtype=np.float64)
    for r in results:
        mm = np.asarray(r["mm_out"], dtype=np.float64)  # [7, P, 129]
        tp += np.trace(mm[:, :, :128], axis1=1, axis2=2)
        pred_cnt += mm[:, :, 128].sum(axis=1)
        ga = np.asarray(r["ga_out"], dtype=np.float64).sum(axis=0)  # [14]
        # each slot's flat accum includes one ones-column entry per block
        # per partition: 16 (img0) + 16 (img1) = 32*P total
        gt_cnt += ga[0:7] + ga[7:14] - 2 * 16 * P

    tp32 = tp.astype(np.float32)
    fp32_ = (pred_cnt - tp).astype(np.float32)
    fn32 = (gt_cnt - tp).astype(np.float32)
    eps = np.float32(EPS)
    two = np.float32(2.0)
    dice = (two * tp32 + eps) / (two * tp32 + fp32_ + fn32 + eps)
    return np.asarray(np.mean(dice, dtype=np.float32), dtype=np.float32)


def kernel(**inputs):
    from concourse.bass_utils import run_bass_kernel_spmd

    nc = _get_bass()
    in_maps = make_in_maps(inputs["y_true"], inputs["y_pred"])
    res = run_bass_kernel_spmd(nc, in_maps, core_ids=list(range(N_CORES)))
    return epilogue(res.results)


if __name__ == "__main__":
    # smoke test with random data
    rng = np.random.default_rng(0)
    y_true = rng.integers(0, C, size=(16, 512, 512)).astype(np.int32)
    y_pred = rng.standard_normal((16, C, 512, 512)).astype(np.float32)
    out = kernel(y_true=y_true, y_pred=y_pred)
    print("kernel output:", out)



# revision 33
# speedup vs baseline: 1.2770x; 1.1509x over previous
"""Trainium2 Bass kernel for DiceLoss (hard-argmax dice, ignore background, mean).

Problem (hardcoded shapes):
  y_true: [16, 512, 512] int32 in [0, 8)
  y_pred: [16, 8, 512, 512] float32
  out   : scalar float32 = mean over classes 1..7 of
          (2*tp + eps) / (2*tp + fp + fn + eps)
  with pred_cls = argmax_c y_pred, one-hot tp/fp/fn sums over all pixels.

Strategy (8 NeuronCores, data-parallel over batch; 2 images per core):
  - Each channel plane is one [128, 2048] tile. y_pred is loaded via SWDGE
    cast-DMA (f32 in HBM -> bf16 in SBUF): HBM read traffic is unchanged but
    every on-chip elementwise op runs in DVE 16-bit perf modes and no
    convert instructions are needed. The per-core stream is a single SWDGE
    FIFO at the HBM bandwidth limit, so everything else is ordered around
    its arrival times: image 0 loads whole planes; image 1 loads two half
    planes so only half a plane of compute trails the final DMA completion.
  - DVE (all bf16, no accum_out so the 2x/4x perf-mode uops stay eligible):
      * 7-op pairwise max tree -> m = max over channels      (2x_1P)
      * pred_c = (ch_c == m) via tensor_tensor is_equal      (2x_1P)
      * gt_c   = (tf == c) via tensor_single_scalar is_equal (4x_2P),
        written strided into a [128, 16, 130] block layout whose col 128
        holds a persistent ones column (memset once). Separate gt tile
        sets per image so image 1's writes never wait on image 0's matmul
        readers (WAR convoy).
  - ScalarE: int32->bf16 label convert; per section a flat copy-with-
    accum_out over the gt block layout that yields the per-partition gt
    counts (host subtracts the constant ones contribution); PSUM evac.
  - TensorE: per class-subtile one matmul with lhsT = pred subtile and
    rhs = [gt subtile | ones] (129 cols) accumulated over subtiles+images:
    diag gives tp, column 128 gives pred counts. Host reads trace + sums.
  - Host: combines the 8 cores' exact-integer f32 partials; dice needs only
    tp and pred_cnt+gt_cnt (denominator = 2tp+fp+fn = pred+gt), formed in
    float32 to match the reference arithmetic.
"""

import numpy as np

EPS = 1e-05

# Problem geometry (hardcoded per the harness contract).
N_CORES = 8
NB = 2            # batch images per core
C = 8             # classes
P = 128           # SBUF partitions
FD = 2048         # free-dim elements per channel plane (512*512 = 128*2048)
NSUB = FD // 128  # 128-wide subtiles per plane for the PE matmuls
BLK = 130         # gt block stride: 128 gt cols + ones col + 1 pad (4B align)

_CACHED_NC = None


def build_bass():
    """Build the Bass kernel (same NEFF for all 8 cores)."""
    from contextlib import ExitStack

    import concourse.bacc as bacc
    import concourse.tile as tile
    from concourse import mybir

    nc = bacc.Bacc(None, target_bir_lowering=False)

    yp = nc.dram_tensor("yp", [NB, C, P, FD], mybir.dt.float32, kind="ExternalInput")
    yt = nc.dram_tensor("yt", [NB, P, FD], mybir.dt.int32, kind="ExternalInput")
    # per class: [128, 129] PSUM accumulator (cross-products + pred colsum).
    mm_out = nc.dram_tensor("mm_out", [7, P, 129], mybir.dt.float32, kind="ExternalOutput")
    # per-partition gt counts: slots 0..6 = (img0, class), 7..13 = (img1,
    # half 0, class), 14..20 = (img1, half 1, class)
    ga_out = nc.dram_tensor("ga_out", [P, 21], mybir.dt.float32, kind="ExternalOutput")

    with tile.TileContext(nc) as tc, ExitStack() as ctx:
        chpool = ctx.enter_context(tc.tile_pool(name="ch", bufs=1))
        tpool = ctx.enter_context(tc.tile_pool(name="tt", bufs=1))
        mpool = ctx.enter_context(tc.tile_pool(name="mx", bufs=2))
        mtmp = ctx.enter_context(tc.tile_pool(name="mtmp", bufs=6))
        predp = ctx.enter_context(tc.tile_pool(name="pred", bufs=5))
        scrp = ctx.enter_context(tc.tile_pool(name="scr", bufs=1))
        accp = ctx.enter_context(tc.tile_pool(name="acc", bufs=1))
        psump = ctx.enter_context(tc.tile_pool(name="psum", bufs=1, space="PSUM"))

        ga_acc = accp.tile([P, 21], mybir.dt.float32, name="ga_acc")
        # fixed per-(image, class) gt tiles in block layout [128, 16, 130]:
        # cols 0:128 = gt mask, col 128 = ones, col 129 = zero pad (so a
        # flat [128, 2080] read sums cleanly). Separate tiles per image so
        # image 1's gt writes never wait on image 0's matmul readers.
        gtsets = [
            [
                accp.tile([P, NSUB, BLK], mybir.dt.bfloat16, name=f"gt{n}_{c}")
                for c in range(1, C)
            ]
            for n in range(NB)
        ]
        psums = [
            psump.tile([P, 129], mybir.dt.float32, name=f"ps{c}", tag=f"ps{c}")
            for c in range(1, C)
        ]

        for gset in gtsets:
            for g in gset:
                nc.vector.memset(g[:, :, 128:129], 1.0)
                nc.vector.memset(g[:, :, 129:130], 0.0)

        HF = FD // 2   # half-plane free dim
        HS = NSUB // 2  # subtiles per half

        # ---- all loads up front: gpsimd queue delivers the casts FIFO in
        # exactly this order; labels ride the concurrent HWDGE queue.
        # Image 0 loads whole planes; image 1 loads half-planes (half-major)
        # so the tail after the last byte is only half a plane's compute. ----
        ch = {}   # (n, c) -> full-plane AP;  (1, c, h) -> half-plane AP
        tf = {}
        for n in range(NB):
            # labels in TWO half loads + converts per image: each convert
            # fires ~2us after its half lands instead of waiting the full
            # 1 MiB DMA's completion, so tf1 is ready ~4us earlier and the
            # img1 gt masks (which gate Vector's early program) start
            # sooner. Touches nothing else in the schedule.
            tfn = tpool.tile([P, FD], mybir.dt.bfloat16, name="tf", tag=f"tf{n}")
            for h in range(2):
                ti = tpool.tile([P, HF], mybir.dt.int32, name="ti", tag=f"ti{n}{h}")
                nc.sync.dma_start(out=ti, in_=yt[n][:, h * HF : (h + 1) * HF])
                # labels to bf16 (exact for 0..7) on ScalarE
                nc.scalar.copy(out=tfn[:, h * HF : (h + 1) * HF], in_=ti)
            tf[n] = tfn
        for c in range(C):
            tl = chpool.tile([P, FD], mybir.dt.bfloat16, name=f"ch{c}", tag=f"n0ch{c}")
            # SWDGE cast-DMA: f32 HBM -> bf16 SBUF
            nc.gpsimd.dma_start(out=tl, in_=yp[0, c])
            ch[0, c] = tl
        # image 1 sections: two half-planes (quarter-splitting the tail was
        # measured slower — the extra small DMAs lengthen the SWDGE FIFO
        # more than the shorter trailing compute chain saves).
        sections = [(0, HF), (HF, HF)]
        im1 = {}
        for c in range(C):
            im1[c] = chpool.tile([P, FD], mybir.dt.bfloat16, name=f"ch{c}", tag=f"n1ch{c}")
        for si, (off, ln) in enumerate(sections):
            for c in range(C):
                part = im1[c][:, off : off + ln]
                nc.gpsimd.dma_start(out=part, in_=yp[1, c][:, off : off + ln])
                ch[1, c, si] = part

        def emit_gt(gts, slot, tfv, c, s0, ns):
            """gt mask (DVE 4x) + gt count (ScalarE flat copy w/ accum).
            Writes subtile blocks s0..s0+ns of class c's gt tile."""
            g = gts[c - 1]
            gv = g[:, s0 : s0 + ns, 0:128]
            nc.vector.tensor_single_scalar(
                out=gv, in_=tfv, scalar=float(c), op=mybir.AluOpType.is_equal
            )
            scr = scrp.tile([P, NSUB * BLK], mybir.dt.bfloat16, name="scr", tag="scr")
            # flat contiguous read (incl. ones + zero pad; host subtracts
            # the constant 16 per partition per block) keeps ScalarE fast
            nc.scalar.activation(
                out=scr[:, 0 : ns * BLK],
                in_=g[:, s0 : s0 + ns, :].rearrange("p s f -> p (s f)"),
                func=mybir.ActivationFunctionType.Copy,
                accum_out=ga_acc[:, slot : slot + 1],
            )

        def emit_tree(chs, fd, t0, dt):
            """Serial-chain max: the last-arriving channels join closest to
            the root so only 1-2 ops trail the final DMA. The tile_wait_until
            stamps tell the static scheduler the real DMA arrival times, so
            it slots the (early-ready) gt-mask ops into the wait windows
            instead of head-of-line blocking the DVE queue on the tree."""
            t1 = mtmp.tile([P, FD], mybir.dt.bfloat16, name="t1", tag="mt")
            with tc.tile_wait_until(t0 + 2 * dt):
                nc.vector.tensor_max(t1[:, 0:fd], chs[0], chs[1])
            t2 = mtmp.tile([P, FD], mybir.dt.bfloat16, name="t2", tag="mt")
            t12 = mtmp.tile([P, FD], mybir.dt.bfloat16, name="t12", tag="mt")
            with tc.tile_wait_until(t0 + 4 * dt):
                nc.vector.tensor_max(t2[:, 0:fd], chs[2], chs[3])
                nc.vector.tensor_max(t12[:, 0:fd], t1[:, 0:fd], t2[:, 0:fd])
            t3 = mtmp.tile([P, FD], mybir.dt.bfloat16, name="t3", tag="mt")
            t123 = mtmp.tile([P, FD], mybir.dt.bfloat16, name="t123", tag="mt")
            with tc.tile_wait_until(t0 + 6 * dt):
                nc.vector.tensor_max(t3[:, 0:fd], chs[4], chs[5])
                nc.vector.tensor_max(t123[:, 0:fd], t12[:, 0:fd], t3[:, 0:fd])
            t6 = mtmp.tile([P, FD], mybir.dt.bfloat16, name="t6", tag="mt")
            with tc.tile_wait_until(t0 + 7 * dt):
                nc.vector.tensor_max(t6[:, 0:fd], t123[:, 0:fd], chs[6])
            m = mpool.tile([P, FD], mybir.dt.bfloat16, name="m", tag="m")
            with tc.tile_wait_until(t0 + 8 * dt):
                nc.vector.tensor_max(m[:, 0:fd], t6[:, 0:fd], chs[7])
            return m

        def emit_pred_mm(gts, chv, m, c, s0, ns, start, stop):
            pred = predp.tile([P, FD], mybir.dt.bfloat16, name=f"pred{c}", tag="pred")
            predv = pred[:, 0 : ns * 128]
            nc.vector.tensor_tensor(
                out=predv, in0=chv, in1=m, op=mybir.AluOpType.is_equal
            )
            g = gts[c - 1]
            for s in range(ns):
                nc.tensor.matmul(
                    psums[c - 1][:, :],
                    lhsT=predv[:, s * 128 : (s + 1) * 128],
                    rhs=g[:, s0 + s, 0:129],
                    start=(start and s == 0),
                    stop=(stop and s == ns - 1),
                )

        # ---- DVE program, ordered to match arrival times ----
        # All gt masks first: they depend only on the labels (arrive within
        # ~12us on the HWDGE queue) so DVE starts productive work ~6us
        # before the first y_pred channel lands.
        # image 0's gt masks run entirely on ScalarE: gt = relu(1 - |tf - c|)
        # with the gt count fused into the Relu's accum_out (which replaces
        # the separate count-copy, so ScalarE gains only one op per class
        # while DVE sheds all seven tensor_scalar ops).
        tf3_0 = tf[0].rearrange("p (s f) -> p s f", s=NSUB)
        for c in range(1, C):
            emit_gt(gtsets[0], c - 1, tf3_0, c, 0, NSUB)
        tf3_1 = tf[1].rearrange("p (s f) -> p s f", s=NSUB)
        for si, (off, ln) in enumerate(sections):
            s0, ns = off // 128, ln // 128
            for c in range(1, C):
                emit_gt(gtsets[1], 7 + si * 7 + (c - 1), tf3_1[:, s0 : s0 + ns, :], c, s0, ns)

        # Measured SWDGE FIFO timing (ms): first bytes ~11.5us in, then one
        # 1 MiB full-plane cast every ~2.8us (half planes ~1.4us).
        T0 = 0.0115
        DT_FULL = 0.0028
        DT_HALF = 0.0014

        # image 0: tree, then pred+MM per class.
        m0 = emit_tree([ch[0, c] for c in range(C)], FD, T0 - DT_FULL, DT_FULL)
        for c in range(1, C):
            emit_pred_mm(gtsets[0], ch[0, c], m0, c, 0, NSUB, start=True, stop=False)

        # image 1, by section: tree + pred/MM.
        for si, (off, ln) in enumerate(sections):
            s0, ns = off // 128, ln // 128
            t0 = T0 + 8 * DT_FULL + si * 8 * DT_HALF - DT_HALF
            mh = emit_tree([ch[1, c, si] for c in range(C)], ln, t0, DT_HALF)
            for c in range(1, C):
                emit_pred_mm(
                    gtsets[1], ch[1, c, si], mh[:, 0:ln], c, s0, ns,
                    start=False, stop=(si == len(sections) - 1),
                )

        nc.sync.dma_start(out=ga_out[:], in_=ga_acc)
        for c in range(7):
            pt = accp.tile([P, 129], mybir.dt.float32, name=f"pt{c}", tag=f"pt{c}")
            nc.scalar.copy(out=pt, in_=psums[c])
            nc.sync.dma_start(out=mm_out[c], in_=pt)

    nc.finalize()
    return nc


def _get_bass():
    global _CACHED_NC
    if _CACHED_NC is None:
        _CACHED_NC = build_bass()
    return _CACHED_NC


def make_in_maps(y_true, y_pred):
    yp = np.ascontiguousarray(np.asarray(y_pred, dtype=np.float32))
    yt = np.ascontiguousarray(np.asarray(y_true, dtype=np.int32))
    in_maps = []
    for i in range(N_CORES):
        yps = np.ascontiguousarray(yp[NB * i : NB * (i + 1)]).reshape(NB, C, P, FD)
        yts = np.ascontiguousarray(yt[NB * i : NB * (i + 1)]).reshape(NB, P, FD)
        in_maps.append({"yp": yps, "yt": yts})
    return in_maps


def epilogue(results):
    """Combine the 8 cores' partial sums into the final dice mean (float32,
    mirroring the reference arithmetic)."""
    tp = np.zeros(7, dtype=np.float64)
    pred_cnt = np.zeros(7, dtype=np.float64)
    gt_cnt = np.zeros(7, dtype=np.float64)
    for r in results:
        mm = np.asarray(r["mm_out"], dtype=np.float64)  # [7, P, 129]
        tp += np.trace(mm[:, :, :128], axis1=1, axis2=2)
        pred_cnt += mm[:, :, 128].sum(axis=1)
        ga = np.asarray(r["ga_out"], dtype=np.float64).sum(axis=0)  # [21]
        # each slot's flat accum includes one ones-column entry per block
        # per partition: 16 (img0) + 8 + 8 (img1 halves) = 32*P total
        gt_cnt += ga[0:7] + ga[7:14] + ga[14:21] - 2 * 16 * P

    tp32 = tp.astype(np.float32)
    fp32_ = (pred_cnt - tp).astype(np.float32)
    fn32 = (gt_cnt - tp).astype(np.float32)
    eps = np.float32(EPS)
    two = np.float32(2.0)
    dice = (two * tp32 + eps) / (two * tp32 + fp32_ + fn32 + eps)
    return np.asarray(np.mean(dice, dtype=np.float32), dtype=np.float32)


def kernel(**inputs):
    from concourse.bass_utils import run_bass_kernel_spmd

    nc = _get_bass()
    in_maps = make_in_maps(inputs["y_true"], inputs["y_pred"])
    res = run_bass_kernel_spmd(nc, in_maps, core_ids=list(range(N_CORES)))
    return epilogue(res.results)


if __name__ == "__main__":
    # smoke test with random data
    rng = np.random.default_rng(0)
    y_true = rng.integers(0, C, size=(16, 512, 512)).astype(np.int32)
    y_pred = rng.standard_normal((16, C, 512, 512)).astype(np.float32)
    out = kernel(y_true=y_true, y_pred=y_pred)
    print("kernel output:", out)

